# revision 92
# baseline (speedup 1.0000x reference)
"""Causal multi-head attention (B=2, S=2048, D=1024, H=16) on 8 trn2
NeuronCores.

Sharding (head-parallel): core c handles batch c//4 and heads
4*(c%4) .. 4*(c%4)+3 (a 256-wide slice of the q/k/v feature dim).  W_proj is
tensor-parallel split along the head dim, so each core emits a full-shape
[S, D] partial projection output; the host sums the 4 partials per batch.

Per-core schedule (bf16 matmul inputs and DMA payloads, f32 PSUM
accumulation), organized to keep the PE dense from first DMA to the last
projection:
  - load phase: 6 q/k/v projection accumulators run chunk-major so each xT
    chunk is consumed the moment it lands (input DMA is the limiter here);
    dummy matmuls on a spare PSUM bank keep the PE p-state ramped between
    chunk arrivals
  - attention runs quarter-major per head-pair (pair 0 ascending, pair 1
    descending), software-pipelined one strip deep (scores j+1 issue before
    AV j) with the exp on the Activation engine trailing one strip behind,
    and each quarter's last AV + normalization deferred into the next
    quarter so the PE never waits on the final exp at a boundary
  - the remaining projection groups, v tiles, and output-projection tiles
    are emitted as filler units between strips (a require() watermark forces
    the queue through a unit before anything consumes its output -- Tile
    does not detect use-before-def); fillers concentrate on the
    latency-bound diagonal strips
  - v tiles carry [64 v-cols | 64 ones-cols] so the AV matmul emits 64
    replicated softmax-denominator rows for free; softmax needs no max
    subtraction (scores ~ N(0,1), exp cannot overflow)
  - causal masking of diagonal blocks: exp first, then the strict-upper
    triangle is zeroed by a bf16 0/1-mask multiply on DVE's 4x path
  - output tiles stage through SBUF bf16 (Act copies one half, DVE the
    other) and DMA out via Pool's SWDGE mid-run / the idle HWDGE at the tail

The TRN2 ISA holds one sync-wait per instruction; Tile emits more, so
excess waits are hoisted onto same-engine NoOps after scheduling.
"""

import os

import numpy as np

# cache compiled executables (incl. the wrapped NEFF) across processes
os.environ.setdefault("JAX_COMPILATION_CACHE_DIR", "/tmp/jax_comp_cache")
os.environ.setdefault("JAX_PERSISTENT_CACHE_MIN_ENTRY_SIZE_BYTES", "0")
os.environ.setdefault("JAX_PERSISTENT_CACHE_MIN_COMPILE_TIME_SECS", "0")

S = 2048
D = 1024
DH = 64
P = 128
NT = S // P   # 16 sequence tiles
DC = D // P   # 8 contraction chunks
N_CORES = 8

WARMUP_DUMMIES = int(os.environ.get("K_W0", 20))  # PE warm-up before chunk 0
CHUNK_DUMMIES = int(os.environ.get("K_W1", 6))   # PE busy between chunks
ET_BUFS = int(os.environ.get("K_ET", 8))
OT_BUFS = int(os.environ.get("K_OT", 8))
CADENCE = int(os.environ.get("K_CAD", 1))         # pr0 filler every Nth strip
BOUNDARY = int(os.environ.get("K_BND", 2))
DIAGC = int(os.environ.get("K_DIA", 1))        # fillers at quarter bound

_CACHE = {}


def _build_bass():
    import concourse.bass as bass
    import concourse.tile as tile
    from concourse import mybir

    f32 = mybir.dt.float32
    bf16 = mybir.dt.bfloat16
    EXP = mybir.ActivationFunctionType.Exp

    nc = bass.Bass("TRN2")

    xT_d = nc.dram_tensor("xT", [D, S], bf16, kind="ExternalInput")
    wqkv_d = nc.dram_tensor("wqkv", [D, 768], bf16, kind="ExternalInput")
    wp_d = nc.dram_tensor("wp_t", [256, D], bf16, kind="ExternalInput")
    m01_d = nc.dram_tensor("m01", [P, 2, P], bf16, kind="ExternalInput")
    out_d = nc.dram_tensor("out", [S, D], bf16, kind="ExternalOutput")

    with tile.TileContext(nc) as tc:
        with tc.tile_pool(name="persist", bufs=1) as persist:
            # q/k in transposed [dh, s] bf16 layout, one tile per 512-col group
            # so interleaved consumers see precise dependencies
            qTn = [[persist.tile([P, 512], bf16, name=f"qT{p}n{n}",
                                 tag=f"qT{p}n{n}") for n in range(4)]
                   for p in range(2)]
            kTn = [[persist.tile([P, 512], bf16, name=f"kT{p}n{n}",
                                 tag=f"kT{p}n{n}") for n in range(4)]
                   for p in range(2)]
            v4e = [persist.tile([P, 4, P], bf16, name=f"v4e{t}", tag=f"v4e{t}")
                   for t in range(NT)]
            wp_sb = [persist.tile([P, D], bf16, name=f"wp{p}", tag=f"wp{p}")
                     for p in range(2)]
            # normalized attention, one tile per (pair, quarter)
            attnq = [[persist.tile([P, 512], bf16, name=f"attn{p}q{q}",
                                   tag=f"attn{p}q{q}") for q in range(4)]
                     for p in range(2)]
            # 0/1 causal mask for diagonal blocks, replicated for both heads;
            # applied to the exp'd weights on the (otherwise idle) Pool engine
            m01_sb = persist.tile([P, 2, P], bf16, name="m01_sb", tag="m01_sb")



            with tc.tile_pool(name="sp", bufs=1, space="PSUM") as sp, \
                 tc.tile_pool(name="xw", bufs=1) as xw:
                xTt = [xw.tile([P, S], bf16, name=f"xTt{c}", tag=f"xTt{c}")
                       for c in range(DC)]
                wqkv_sb = [xw.tile([P, 768], bf16, name=f"wqkv{c}",
                                   tag=f"wqkv{c}") for c in range(DC)]

                # warm-up operand with no input dependency: the PE can start
                # ramping its clock before the first DMA lands.  This memset
                # must be DVE's FIRST op; the v4e ones-memsets follow.
                zero_sb = xw.tile([P, P], bf16, name="zero_sb", tag="zero_sb")
                nc.vector.memset(zero_sb[:], 0.0)
                for t in range(NT):
                    nc.vector.memset(v4e[t][:, :, 64:P], 1.0)

                # DMA issue order: chunk 0 first (earliest PE work), then the
                # tiny mask/ident, remaining chunks, wp last (projection only)
                nc.sync.dma_start(out=xTt[0][:], in_=xT_d[0:P, :])
                nc.sync.dma_start(out=wqkv_sb[0][:], in_=wqkv_d[0:P, :])
                nc.sync.dma_start(out=m01_sb[:], in_=m01_d[:])
                for c in range(1, DC):
                    nc.sync.dma_start(out=xTt[c][:],
                                      in_=xT_d[c * P:(c + 1) * P, :])
                    nc.sync.dma_start(out=wqkv_sb[c][:],
                                      in_=wqkv_d[c * P:(c + 1) * P, :])
                for p in range(2):
                    nc.sync.dma_start(out=wp_sb[p][:],
                                      in_=wp_d[p * P:(p + 1) * P, :])

                def qk_lhsT(c, which, p):
                    base = 0 if which == "q" else 256
                    return wqkv_sb[c][:, base + p * P:base + (p + 1) * P]

                def copy_qk(ps, which, p, n, eng="v"):
                    dst = (qTn if which == "q" else kTn)[p][n]
                    if eng == "v":
                        nc.vector.tensor_copy(dst[:], ps[:])
                    else:
                        nc.scalar.copy(dst[:], ps[:])

                def copy_v(ps, t):
                    nc.vector.tensor_copy(
                        v4e[t][:, :, 0:64],
                        ps[:, 0:256].rearrange("p (h d) -> p h d", h=4))

                # ---------- load phase: chunk-major over 6 accumulators ------
                # v0/v1 accumulate in the sp banks (not the load pool) so the
                # strip pool's alloc only waits on the four q/k close-copies
                spctr = [0]

                def sp_tile():
                    t = sp.tile([P, 512], f32, name="spt",
                                tag=f"sp{spctr[0] & 1}", bufs=1)
                    spctr[0] += 1
                    return t

                with tc.tile_pool(name="load", bufs=1, space="PSUM") as lp:
                    lq = [lp.tile([P, 512], f32, name=f"lq{n}", tag=f"lq{n}")
                          for n in range(2)]           # q0n0, q0n1
                    lk = [lp.tile([P, 512], f32, name=f"lk{n}", tag=f"lk{n}")
                          for n in range(2)]           # k0n0, k0n1
                    lv = [lp.tile([P, 512], f32, name=f"lv{t}", tag=f"lv{t}")
                          for t in range(2)]           # v0, v1

                    # dummies live in an sp bank (idle during load): sharing a
                    # bank with a live accumulation corrupts it
                    dt = sp_tile()

                    def dummy(k):
                        for _ in range(k):
                            nc.tensor.matmul(
                                dt[:, 0:P], lhsT=zero_sb[:],
                                rhs=zero_sb[:], start=True, stop=True,
                                skip_group_check=True)

                    dummy(WARMUP_DUMMIES)
                    for c in range(DC):
                        st, sp_ = (c == 0), (c == DC - 1)
                        for n in range(2):
                            nc.tensor.matmul(
                                lq[n][:], lhsT=qk_lhsT(c, "q", 0),
                                rhs=xTt[c][:, n * 512:(n + 1) * 512],
                                start=st, stop=sp_)
                            nc.tensor.matmul(
                                lk[n][:], lhsT=qk_lhsT(c, "k", 0),
                                rhs=xTt[c][:, n * 512:(n + 1) * 512],
                                start=st, stop=sp_)
                        for t in range(2):
                            nc.tensor.matmul(
                                lv[t][:, 0:256],
                                lhsT=xTt[c][:, t * P:(t + 1) * P],
                                rhs=wqkv_sb[c][:, 512:768],
                                start=st, stop=sp_, skip_group_check=True)
                        if c < DC - 1:
                            dummy(CHUNK_DUMMIES)
                    # k copies on Activation (idle here) so the first strip's
                    # q and k land simultaneously; v copies (sp banks) follow
                    # outside the pool-release path
                    copy_qk(lq[0], "q", 0, 0)
                    copy_qk(lk[0], "k", 0, 0, eng="s")
                    copy_v(lv[0], 0)
                    nc.scalar.copy(
                        v4e[1][:, :, 0:64],
                        lv[1][:, 0:256].rearrange("p (h d) -> p h d", h=4))
                    copy_qk(lq[1], "q", 0, 1)
                    copy_qk(lk[1], "k", 0, 1, eng="s")

                # ---------- filler units for the attention phase -------------
                # each unit may carry a `provides` key; consumers call
                # require(key) to force-drain the queue through the producing
                # unit, so emission order always satisfies data flow (Tile
                # does NOT catch use-before-def)
                fillers = []
                produced = {("q", 0, 0), ("k", 0, 0), ("q", 0, 1),
                            ("k", 0, 1), ("v", 0), ("v", 1)}

                def mk_v_unit(t):
                    def u():
                        ps = sp_tile()
                        for c in range(DC):
                            nc.tensor.matmul(
                                ps[:, 0:256],
                                lhsT=xTt[c][:, t * P:(t + 1) * P],
                                rhs=wqkv_sb[c][:, 512:768],
                                start=(c == 0), stop=(c == DC - 1),
                                skip_group_check=True)
                        copy_v(ps, t)
                    u.provides = ("v", t)
                    return [u]

                def mk_qk_units(which, p, n):
                    st = {}

                    def u(cs):
                        def f():
                            if cs[0] == 0:
                                st["t"] = sp_tile()
                            for c in cs:
                                nc.tensor.matmul(
                                    st["t"][:], lhsT=qk_lhsT(c, which, p),
                                    rhs=xTt[c][:, n * 512:(n + 1) * 512],
                                    start=(c == 0), stop=(c == DC - 1))
                            if cs[-1] == DC - 1:
                                copy_qk(st["t"], which, p, n)
                        if cs[-1] == DC - 1:
                            f.provides = (which, p, n)
                        return f
                    return [u([0, 1]), u([2, 3]), u([4, 5]), u([6, 7])]

                fillers += mk_v_unit(2) + mk_v_unit(3)
                fillers += mk_v_unit(4) + mk_v_unit(5) + mk_v_unit(6) \
                    + mk_v_unit(7)
                fillers += mk_qk_units("q", 0, 2) + mk_qk_units("k", 0, 2)
                fillers += mk_v_unit(8) + mk_v_unit(9) + mk_v_unit(10) \
                    + mk_v_unit(11)
                fillers += mk_qk_units("q", 0, 3) + mk_qk_units("k", 0, 3)
                fillers += mk_v_unit(12) + mk_v_unit(13) + mk_v_unit(14) \
                    + mk_v_unit(15)
                # pair-1 groups ordered for pr1's descending quarters
                fillers += mk_qk_units("k", 1, 0) + mk_qk_units("k", 1, 1)
                fillers += mk_qk_units("k", 1, 2) + mk_qk_units("k", 1, 3)
                fillers += mk_qk_units("q", 1, 3) + mk_qk_units("q", 1, 2)
                fillers += mk_qk_units("q", 1, 1) + mk_qk_units("q", 1, 0)

                fq = {"i": 0}

                def consume(k):
                    while k > 0 and fq["i"] < len(fillers):
                        u = fillers[fq["i"]]
                        fq["i"] += 1
                        u()
                        key = getattr(u, "provides", None)
                        if key is not None:
                            produced.add(key)
                        k -= 1

                def require(key):
                    while key not in produced:
                        assert fq["i"] < len(fillers), f"no producer for {key}"
                        consume(1)

                # ---------- attention + interleaved projection ---------------
                proj_psum = [sp_tile]

                with tc.tile_pool(name="att", bufs=2) as att:

                    ots = {}

                    def mk_proj_unit(t, oc, tail=False, last=False):
                        def u():
                            pso = proj_psum[0]()
                            for p in range(2):
                                nc.tensor.matmul(
                                    pso[:],
                                    lhsT=attnq[p][t // 4][
                                        :, (t % 4) * P:(t % 4 + 1) * P],
                                    rhs=wp_sb[p][:, oc * 512:(oc + 1) * 512],
                                    start=(p == 0), stop=(p == 1))
                            if last:
                                # final quarter: per-half copies + DMAs so the
                                # very last transfer is as small as possible
                                ot = att.tile([P, 512], bf16, name="ot2",
                                              tag="ot2", bufs=4)
                                if oc == 0:
                                    nc.scalar.copy(ot[:], pso[:])
                                else:
                                    nc.vector.tensor_copy(ot[:], pso[:])
                                nc.sync.dma_start(
                                    out=out_d[t * P:(t + 1) * P,
                                              oc * 512:(oc + 1) * 512],
                                    in_=ot[:])
                            elif oc == 0:
                                ots[t] = att.tile([P, D], bf16, name="ot",
                                                  tag="ot", bufs=OT_BUFS)
                                nc.scalar.copy(ots[t][:, 0:512], pso[:])
                            else:
                                # second half on DVE (halves split across the
                                # two copy engines), then one whole-tile DMA
                                # through the idle Pool engine's SWDGE
                                ot = ots.pop(t)
                                nc.vector.tensor_copy(ot[:, 512:D], pso[:])
                                if tail:
                                    # HWDGE is idle at the tail; Pool SWDGE
                                    # issue (1038ns) would serialize here
                                    nc.sync.dma_start(
                                        out=out_d[t * P:(t + 1) * P, :],
                                        in_=ot[:])
                                else:
                                    nc.gpsimd.dma_start(
                                        out=out_d[t * P:(t + 1) * P, :],
                                        in_=ot[:])
                        return [u]

                    # pr0 ascending (v tiles arrive as fillers just in time);
                    # pr1 descending so its projection tiles become filler for
                    # the later quarters and the tail quarter is the smallest
                    # pa pool outlives the strip pool: the drain pool below
                    # reuses the strip banks while the last norm (a pa
                    # reader) is still in flight
                    ps_a_cm = tc.tile_pool(name="ps_a", bufs=1, space="PSUM")
                    ps_a = ps_a_cm.__enter__()
                    ps_s_cm = tc.tile_pool(name="ps_s", bufs=2, space="PSUM")
                    ps_s = ps_s_cm.__enter__()
                    pending = [None]

                    for pr, qcs in ((0, (0, 1, 2, 3)), (1, (3, 2, 1, 0))):
                        for qc in qcs:               # 512-col sq quarter
                            c0 = qc * 512
                            pa = [ps_a.tile([P, 512], f32, name=f"pa{h}",
                                            tag=f"pa{h}", bufs=1)
                                  for h in range(2)]
                            jmax = min(4 * qc + 3, NT - 1)
                            ets = {}

                            def emit_scores(j, pr=pr, qc=qc, c0=c0,
                                            ets=ets):
                                require(("q", pr, qc))
                                require(("k", pr, j // 4))
                                w0 = j * P
                                lo = max(w0, c0)
                                w = c0 + 512 - lo
                                strip = ps_s.tile([P, 1024], f32,
                                                  name="strip", tag="strip")
                                for h in range(2):
                                    nc.tensor.matmul(
                                        strip[:, h * 512 + lo - c0:
                                              h * 512 + lo - c0 + w],
                                        lhsT=kTn[pr][j // 4][
                                            h * 64:(h + 1) * 64,
                                            w0 - (j // 4) * 512:
                                            w0 - (j // 4) * 512 + P],
                                        rhs=qTn[pr][qc][h * 64:(h + 1) * 64,
                                                        lo - c0:lo - c0 + w],
                                        start=True, stop=True,
                                        skip_group_check=True)
                                et = att.tile([P, 1024], bf16, name="et",
                                              tag="et", bufs=ET_BUFS)
                                sv = strip.rearrange("p (h q) -> p h q", h=2)
                                ev = et.rearrange("p (h q) -> p h q", h=2)
                                nc.scalar.activation(
                                    out=ev[:, :, lo - c0:lo - c0 + w],
                                    in_=sv[:, :, lo - c0:lo - c0 + w],
                                    func=EXP)
                                if j // 4 == qc:
                                    # zero the strict-upper triangle of the
                                    # diagonal block (both heads); bf16 SBUF
                                    # operands hit DVE's 4x mode (~190ns)
                                    dv = ev[:, :, w0 - c0:w0 - c0 + P]
                                    nc.vector.tensor_mul(dv, dv, m01_sb[:])
                                ets[j] = (et, lo, w)

                            def emit_av(j, pr=pr, c0=c0, jmax=jmax,
                                        pa=pa, ets=ets):
                                require(("v", j))
                                et, lo, w = ets.pop(j)
                                for h in range(2):
                                    nc.tensor.matmul(
                                        pa[h][:, lo - c0:lo - c0 + w],
                                        lhsT=v4e[j][:, 2 * pr + h, :],
                                        rhs=et[:, h * 512 + lo - c0:
                                               h * 512 + lo - c0 + w],
                                        start=(j == 0), stop=(j == jmax),
                                        skip_group_check=True)

                            for j in range(jmax + 1):
                                emit_scores(j)
                                # full strips are PE-bound; diagonal strips
                                # (shrinking width) are latency-bound and need
                                # the filler cover, especially in pr1 where
                                # supply is scarce
                                if pr == 1:
                                    if j // 4 == qc:
                                        consume(DIAGC)
                                elif j % CADENCE == CADENCE - 1:
                                    consume(1)
                                if j == 1 and pending[0] is not None:
                                    # close the PREVIOUS quarter here: its
                                    # last exp had two scores' worth of time
                                    # to finish, so the deferred AV does not
                                    # stall the PE
                                    pending[0]()
                                    pending[0] = None
                                if j > 0:
                                    emit_av(j - 1)

                            def close_quarter(pa=pa, jmax=jmax,
                                              pr=pr, qc=qc,
                                              emit_av=emit_av):
                                consume(1)
                                emit_av(jmax)
                                # r0,m0,r1,m1: mul h0 must not queue behind
                                # recip h1 on DVE -- the next quarter's first
                                # AV only waits on pa[h0]'s readers
                                for h in range(2):
                                    recip = att.tile([64, 512], f32,
                                                     name="recip",
                                                     tag="recip", bufs=4)
                                    nc.vector.reciprocal(recip[:],
                                                         pa[h][64:P, :])
                                    nc.vector.tensor_mul(
                                        attnq[pr][qc][h * 64:(h + 1) * 64, :],
                                        pa[h][0:64, :],
                                        recip[:],
                                    )
                                if pr == 1:
                                    tail = qc <= 2
                                    for t in range(4 * qc, 4 * qc + 4):
                                        for oc in range(2):
                                            fillers.extend(
                                                mk_proj_unit(t, oc,
                                                             tail=tail))
                                consume(BOUNDARY)

                            pending[0] = close_quarter

                    pending[0]()   # close the final quarter

                    # strips released (last exp done) but pa still alive: the
                    # drain pool takes the strip banks so tail projections'
                    # pair-0 halves run during the final normalization
                    ps_s_cm.__exit__(None, None, None)
                    with tc.tile_pool(name="ps_o", bufs=4,
                                      space="PSUM") as ps_o:
                        drain_i = [0]

                        def drain_tile():
                            # rotate through 6 banks: 4 ps_o + the 2 sp banks
                            i = drain_i[0]
                            drain_i[0] += 1
                            if i % 3 < 2:
                                return ps_o.tile([P, 512], f32, name="pso",
                                                 tag="pso", bufs=4)
                            return sp_tile()

                        proj_psum[0] = drain_tile
                        consume(len(fillers))
                    ps_a_cm.__exit__(None, None, None)

    return nc


def _fix_matmul_waits(nc):
    """The TRN2 ISA events struct holds exactly ONE sync-wait per
    instruction and walrus codegen refuses instructions carrying more
    ("Too many sync wait commands").  Tile emits multi-wait instructions,
    so legalize: hoist excess waits onto single-wait NoOps inserted right
    before the instruction on the same engine -- engine FIFO order
    preserves the synchronization semantics."""
    import bass_rust
    import concourse.mybir as mybir

    n = 0
    for bb in nc.main_func.blocks:
        insts = bb.instructions
        i = 0
        while i < len(insts):
            ins = insts[i]
            si = getattr(ins, "sync_info", None)
            if si is not None and len(si.on_wait) >= 2:
                for w in si.on_wait[:-1]:
                    nop = mybir.InstNoOp(name=f"I-xwait-{n}", ins=[], outs=[])
                    nop.engine = ins.engine
                    nop.sync_info = bass_rust.SyncInfo(
                        on_wait=[w], on_update=[])
                    insts.insert(i, nop)
                    n += 1
                    i += 1
                ins.sync_info = bass_rust.SyncInfo(
                    on_wait=[si.on_wait[-1]], on_update=si.on_update)
            i += 1
    return n


def get_nc(legalize=True):
    key = ("nc", legalize)
    if key not in _CACHE:
        nc = _build_bass()
        if legalize:
            _fix_matmul_waits(nc)
        _CACHE[key] = nc
    return _CACHE[key]


def make_in_maps(x, W_q, W_k, W_v, W_proj):
    import ml_dtypes

    bf16 = ml_dtypes.bfloat16
    x = np.asarray(x, np.float32)
    W_q = np.asarray(W_q, np.float32)
    W_k = np.asarray(W_k, np.float32)
    W_v = np.asarray(W_v, np.float32)
    W_proj = np.asarray(W_proj, np.float32)

    # 0/1 mask for the diagonal block: weight (sk=p, sq=q) survives iff q >= p
    m01 = np.triu(np.ones((P, P), np.float32)).astype(bf16)
    m01 = np.ascontiguousarray(
        np.broadcast_to(m01[:, None, :], (P, 2, P)))

    xTs = [np.ascontiguousarray(x[b].T).astype(bf16) for b in range(2)]
    in_maps = []
    for core in range(N_CORES):
        b = core // 4
        g = core % 4
        rs = slice(g * 256, (g + 1) * 256)
        wqkv = np.concatenate(
            [W_q[rs].T / 8.0, W_k[rs].T, W_v[rs].T], axis=1).astype(bf16)
        in_maps.append({
            "xT": xTs[b],
            "wqkv": np.ascontiguousarray(wqkv),
            "wp_t": np.ascontiguousarray(W_proj[:, rs].T).astype(bf16),
            "m01": m01,
        })
    return in_maps


def kernel(x, W_q, W_k, W_v, W_proj, _results_hook=None):
    from concourse.bass_utils import run_bass_kernel_spmd

    nc = get_nc()
    in_maps = make_in_maps(x, W_q, W_k, W_v, W_proj)
    res = run_bass_kernel_spmd(nc, in_maps, core_ids=list(range(N_CORES)))
    if _results_hook is not None:
        _results_hook(res)
    out = np.zeros((2, S, D), np.float32)
    for core in range(N_CORES):
        out[core // 4] += res.results[core]["out"].astype(np.float32)
    return out


if __name__ == "__main__":
    nc = get_nc()
    print("built ok; instructions:",
          sum(len(bb.instructions) for bb in nc.main_func.blocks))
